# revision 65
# baseline (speedup 1.0000x reference)
"""Trainium2 8-core transformer layer — batch x head-group sharding.

Core c = (b, j) with b = c // 2, j = c % 2 handles batch b and head
group j (16 of 32 heads), and owns token half j of batch b (512 tokens)
for the residual / MLP stream.

- LN1 computed locally over the full batch's 1024 tokens (duplicated in
  the pair — cheaper than exchanging x1); unit-LN fast path skips the
  weight/bias application when ln_w==1, ln_b==0.
- QKV with streamed Q/K weight packs; V computed transposed (token-major)
  for the ctx matmuls.
- Attention fully local (16 heads x own batch), causal-diagonal aware:
  scores/exp/ctx/denominator matmuls shrink to the valid query range of
  diagonal blocks and only the single 128x128 triangular sub-block is
  masked.  Softmax denominators: e-tiles are elementwise-accumulated on
  DVE across kt blocks (off the ctx critical path), then one [1,TC]
  ones-matmul per head into 32-strided psum rows; one [97,TC] DVE
  reciprocal per head-quad.
- Attention dense projection: row-parallel over the pair; the hh=0 token
  half's chunks are interleaved into attention qc=1 (one spare psum
  bank), then two pairwise bf16 ReduceScatters (one per feature half)
  give each core the summed attn_out for its own 512 tokens.  These are
  the ONLY data collectives in the kernel.
- LN3 + residual + LN2 + full MLP (h4h, gelu, 4hh) + LN4 + final
  residual all fully local on the own 512 tokens; MLP weights (full
  W_h4h / W_4hh) are streamed from HBM in m-chunk packs at ~150 GB/s
  under 99%-busy TensorE.

Measured: 1.147 ms HW exec vs 1.89 ms for the v8 Megatron-style
tensor-parallel baseline; rel err 5.6e-3.
"""

import os
import sys

sys.path.insert(0, "/opt/trn_rl_repo")
os.environ.setdefault("MYCRO_LOCAL_CACHE", "1")
os.environ.setdefault("JAX_PLATFORMS", "cpu,axon")

import numpy as np
import ml_dtypes

import concourse.bass as bass
import concourse.mybir as mybir
import concourse.tile as tile
from concourse import bacc
from concourse.bass_utils import run_bass_kernel_spmd

F32 = mybir.dt.float32
BF16 = mybir.dt.bfloat16
AF = mybir.ActivationFunctionType
ALU = mybir.AluOpType

P = 128
B, S, H, NH = 4, 1024, 2048, 32
HD = H // NH
NC = 8
HG = NH // 2                   # 16 heads per core
NPR = HG // 2                  # 8 head pairs per core
TOK = 512                      # own tokens per core
SB = 1024                      # batch tokens
TC = 512
NFC = H // P                   # 16
NM = 4 * H // P                # 64 inter chunks
F4 = 4 * H
EPS = 1e-5
RG_PAIR = [[0, 1], [2, 3], [4, 5], [6, 7]]

bf16 = ml_dtypes.bfloat16


def _causal_block_status(mask2d):
    mt = mask2d.T
    status = {}
    for kt in range(S // P):
        for qc in range(S // TC):
            blk = mt[kt * P:(kt + 1) * P, qc * TC:(qc + 1) * TC]
            if np.all(blk == 0):
                status[(kt, qc)] = "skip"
            elif np.all(blk == 1):
                status[(kt, qc)] = "full"
            else:
                status[(kt, qc)] = "masked"
    return status


def build_program(block_status, zero_bv=True, zero_bias=True, ln_unit=True):
    nc = bacc.Bacc("TRN2", target_bir_lowering=False, debug=False,
                   num_devices=NC)

    def register_const_ap(dtype, value):
        t = nc.alloc_sbuf_tensor(f"const-{dtype.name}-{value}", [128, 1], dtype)
        nc.gpsimd.memset(t.ap(), value)
        nc.const_aps.aps[(dtype, value)] = t.ap()

    register_const_ap(F32, EPS)
    register_const_ap(F32, float(1.0 / np.sqrt(HD)))
    nc.all_engine_barrier()

    # ---------------- DRAM I/O ----------------
    h_batch = nc.dram_tensor("h_batch", [H, SB], BF16, kind="ExternalInput")
    h_own = nc.dram_tensor("h_own", [H, TOK], BF16, kind="ExternalInput")
    # cpack columns: ln1w 0:16, ln1b 16:32, ln2w 32:48, ln2b 48:64,
    # ln3w 64:80, ln3b 80:96, ln4w 96:112, ln4b 112:128,
    # b_qk 128:144 (m-chunks), b_h4h 144:208 (m), b_4hh 208:224,
    # b_dense 224:240
    cpack_d = nc.dram_tensor("cpack", [P, 240], F32, kind="ExternalInput")
    b_v = nc.dram_tensor("b_v", [1, HG * HD], F32, kind="ExternalInput")
    wqk_d = nc.dram_tensor("wqk", [P, 16 * NFC * P], BF16,
                           kind="ExternalInput")
    wv_d = nc.dram_tensor("wv", [P, 2 * NFC * TC], BF16,
                          kind="ExternalInput")
    wd_d = nc.dram_tensor("wd", [P, NFC * 8 * P], BF16, kind="ExternalInput")
    w4h_d = nc.dram_tensor("w4h", [P, NM * NFC * P], BF16,
                           kind="ExternalInput")
    w4hh_d = nc.dram_tensor("w4hh", [P, NFC * NM * P], BF16,
                            kind="ExternalInput")
    mask_d = nc.dram_tensor("maskb", [P, P], BF16, kind="ExternalInput")
    out_ext = nc.dram_tensor("out", [H, TOK], F32, kind="ExternalOutput")

    masked_blocks = sorted(k for k, v in block_status.items()
                           if v == "masked")
    mask_slot = {blk: i for i, blk in enumerate(masked_blocks)}
    assert len(masked_blocks) <= 8

    with tile.TileContext(nc) as tc:
        with tc.tile_pool(name="const", bufs=1) as const, \
             tc.tile_pool(name="resid", bufs=1) as resid, \
             tc.tile_pool(name="dram", bufs=1, space="DRAM") as dram:

            # ---------- constants ----------
            ones_bf = const.tile([P, 1], BF16)
            nc.vector.memset(ones_bf[:, :], 1.0)
            ones_rows_bf = const.tile([P, P], BF16)
            nc.vector.memset(ones_rows_bf[:, :], 1.0)

            cpack = const.tile([P, 240], F32)
            nc.sync.dma_start(out=cpack[:, :], in_=cpack_d[:, :])
            ln1w, ln1b = cpack[:, 0:16], cpack[:, 16:32]
            ln2w, ln2b = cpack[:, 32:48], cpack[:, 48:64]
            ln3w, ln3b = cpack[:, 64:80], cpack[:, 80:96]
            ln4w, ln4b = cpack[:, 96:112], cpack[:, 112:128]
            bqk = cpack[:, 128:144]
            bh4h = cpack[:, 144:208]
            b4hh = cpack[:, 208:224]
            bdense = cpack[:, 224:240]

            if not zero_bv:
                bv_row = const.tile([1, HG * HD], F32)
                nc.sync.dma_start(out=bv_row[:, :], in_=b_v[0:1, :])
                bv_b = const.tile([P, HG * HD], F32)
                nc.gpsimd.partition_broadcast(bv_b[:, :], bv_row[:, :])

            tri_sb = const.tile([P, P], BF16)
            nc.sync.dma_start(out=tri_sb[:, :], in_=mask_d[:, :])

            # ---------- DRAM bounces ----------
            rs_in = [dram.tile([2 * P, 8 * TC], BF16, name=f"rs_in{i}")
                     for i in range(2)]
            rs_out = [dram.tile([P, 8 * TC], BF16, name=f"rs_out{i}")
                      for i in range(2)]
            warm_in = dram.tile([2, 64], BF16, name="warm_in")
            warm_out = dram.tile([1, 64], BF16, name="warm_out")
            warm_sb = const.tile([2, 64], BF16)
            nc.vector.memset(warm_sb[:, :], 0.0)
            nc.sync.dma_start(out=warm_in[:, :], in_=warm_sb[:, :])
            nc.gpsimd.collective_compute(
                "ReduceScatter", ALU.add, replica_groups=RG_PAIR,
                ins=[warm_in[:, :].opt()], outs=[warm_out[:, :].opt()])

            # ---------- residents ----------
            h_res = resid.tile([P, NFC * TOK], BF16, name="h_res")
            for fc in range(NFC):
                nc.sync.dma_start(out=h_res[:, fc * TOK:(fc + 1) * TOK],
                                  in_=h_own[fc * P:(fc + 1) * P, :])
            ln_in = resid.tile([P, NFC * TOK], BF16, name="ln_in")
            mlp_sb = resid.tile([P, NFC * TOK], BF16, name="mlp_sb")

            # =========================================================
            # Phase 1+2 per token-half q: LN1 -> x1; QK (streamed w);
            # V (streamed w, transposed into v_sb)
            # =========================================================
            p1_cm = tc.tile_pool(name="p1", bufs=1)
            p1 = p1_cm.__enter__()
            x1 = p1.tile([P, NFC * SB], BF16, name="x1")
            qT = p1.tile([P, NPR * SB], BF16, name="qT")
            kT = p1.tile([P, NPR * SB], BF16, name="kT")
            v_sb = p1.tile([P, 8 * HG * HD], BF16, name="v_sb")
            ctxF = p1.tile([P, NPR * SB], BF16, name="ctxF")

            with tc.tile_pool(name="ph1", bufs=1) as ph1, \
                 tc.tile_pool(name="ph1ps", bufs=1, space="PSUM") as ph1ps:
                for q in range(2):
                    # ---- LN1 for tokens q*512..(q+1)*512 ----
                    hq = [ph1.tile([P, TC], BF16, tag=f"hq{fc}", bufs=1,
                                   name=f"hq{fc}") for fc in range(NFC)]
                    for fc in range(NFC):
                        nc.sync.dma_start(
                            out=hq[fc][:, :],
                            in_=h_batch[fc * P:(fc + 1) * P,
                                        q * TC:(q + 1) * TC])
                    ps_s = ph1ps.tile([1, TC], F32, tag="st_s", bufs=1,
                                      name="ps_s")
                    ps_q = ph1ps.tile([1, TC], F32, tag="st_q", bufs=1,
                                      name="ps_q")
                    for fc in range(NFC):
                        nc.tensor.matmul(ps_s[:, :], ones_bf[:, 0:1],
                                         hq[fc][:, :], start=(fc == 0),
                                         stop=(fc == NFC - 1))
                        sq = ph1.tile([P, TC], BF16, tag="sq", bufs=3,
                                      name="sq")
                        nc.vector.tensor_mul(sq[:, :], hq[fc][:, :],
                                             hq[fc][:, :])
                        nc.tensor.matmul(ps_q[:, :], ones_bf[:, 0:1],
                                         sq[:, :], start=(fc == 0),
                                         stop=(fc == NFC - 1))
                    a_b, b2_b = _ln_rows(nc, ph1, ps_s, ps_q, f"ln1q{q}", H)
                    for fc in range(NFC):
                        t1 = ph1.tile([P, TC], F32, tag="t1", bufs=2,
                                      name="t1")
                        nc.vector.tensor_mul(t1[:, :], hq[fc][:, :],
                                             a_b[:, :])
                        xsl = x1[:, fc * SB + q * TC:fc * SB + (q + 1) * TC]
                        if ln_unit:
                            nc.vector.tensor_add(xsl, t1[:, :], b2_b[:, :])
                        else:
                            nc.vector.tensor_add(t1[:, :], t1[:, :],
                                                 b2_b[:, :])
                            nc.vector.tensor_scalar(
                                xsl, t1[:, :], ln1w[:, fc:fc + 1],
                                ln1b[:, fc:fc + 1], ALU.mult, ALU.add)

                    # ---- QK for this half ----
                    for m in range(16):
                        wt = ph1.tile([P, NFC * P], BF16, tag="wqk", bufs=3,
                                      name="wqk")
                        nc.sync.dma_start(
                            out=wt[:, :],
                            in_=wqk_d[:, m * NFC * P:(m + 1) * NFC * P])
                        ps = ph1ps.tile([P, TC], F32, tag="qk", bufs=2,
                                        name="ps_qk")
                        for fc in range(NFC):
                            nc.tensor.matmul(
                                ps[:, :], wt[:, fc * P:(fc + 1) * P],
                                x1[:, fc * SB + q * TC:fc * SB + (q + 1) * TC],
                                start=(fc == 0), stop=(fc == NFC - 1))
                        dst = qT if m < 8 else kT
                        pr = m % 8
                        off = pr * SB + q * TC
                        if zero_bias:
                            nc.scalar.activation(dst[:, off:off + TC],
                                                 ps[:, :], AF.Copy)
                        else:
                            nc.scalar.activation(dst[:, off:off + TC],
                                                 ps[:, :], AF.Identity,
                                                 bias=bqk[:, m:m + 1])

                    # ---- V for this half (psum per 128-token block) ----
                    for vf in range(2):
                        psv = [ph1ps.tile([P, TC], F32, tag=f"v{tt}", bufs=1,
                                          name=f"ps_v{tt}")
                               for tt in range(4)]
                        for fc in range(NFC):
                            wvt = ph1.tile([P, TC], BF16, tag="wv", bufs=3,
                                           name="wvt")
                            nc.sync.dma_start(
                                out=wvt[:, :],
                                in_=wv_d[:, (vf * NFC + fc) * TC:
                                         (vf * NFC + fc + 1) * TC])
                            for tt in range(4):
                                nc.tensor.matmul(
                                    psv[tt][:, :],
                                    x1[:, fc * SB + q * TC + tt * P:
                                       fc * SB + q * TC + (tt + 1) * P],
                                    wvt[:, :],
                                    start=(fc == 0), stop=(fc == NFC - 1))
                        for tt in range(4):
                            voff = (q * 4 + tt) * HG * HD + vf * TC
                            if zero_bv:
                                nc.scalar.activation(v_sb[:, voff:voff + TC],
                                                     psv[tt][:, :], AF.Copy)
                            else:
                                nc.vector.tensor_add(
                                    v_sb[:, voff:voff + TC], psv[tt][:, :],
                                    bv_b[:, vf * TC:(vf + 1) * TC])
            # =========================================================
            # Phase 3: attention, 16 heads in 4 quad-groups
            # =========================================================
            with tc.tile_pool(name="ph3", bufs=1) as ph3, \
                 tc.tile_pool(name="ph3ps", bufs=1, space="PSUM") as ph3ps:

                def dense_chunk(fco, hh):
                    wt = ph3.tile([P, 8 * P], BF16, tag="wd", bufs=3,
                                  name="wd")
                    nc.sync.dma_start(
                        out=wt[:, :],
                        in_=wd_d[:, fco * 8 * P:(fco + 1) * 8 * P])
                    ps = ph3ps.tile([P, TC], F32, tag="d", bufs=1,
                                    name="ps_d")
                    for kc in range(8):
                        nc.tensor.matmul(
                            ps[:, :], wt[:, kc * P:(kc + 1) * P],
                            ctxF[:, kc * SB + hh * TC:
                                 kc * SB + (hh + 1) * TC],
                            start=(kc == 0), stop=(kc == 7))
                    db = ph3.tile([P, TC], BF16, tag="db", bufs=4,
                                  name="db")
                    nc.scalar.activation(db[:, :], ps[:, :], AF.Copy)
                    fh, fr = fco // 8, fco % 8
                    nc.sync.dma_start(
                        out=rs_in[fh][hh * P:(hh + 1) * P,
                                      fr * TC:(fr + 1) * TC],
                        in_=db[:, :])

                for qc in range(2):
                    kts = [kt for kt in range(S // P)
                           if block_status[(kt, qc)] != "skip"]
                    nkt = len(kts)
                    for hg in range(4):
                        ctx_ps = [ph3ps.tile([P, TC], F32, tag=f"ctx{p2}",
                                             bufs=1, name=f"ctx_ps{p2}")
                                  for p2 in range(2)]
                        den_ps = ph3ps.tile([P, TC], F32, tag="den", bufs=1,
                                            name="den_ps")
                        eacc = [ph3.tile([P, TC], BF16, tag=f"eacc{i}",
                                         bufs=1, name=f"eacc{i}")
                                for i in range(4)]

                        def emit_scores(ki):
                            kt = kts[ki]
                            d = kt - qc * (TC // P)
                            lo = max(d, 0) * P
                            es = []
                            for i in range(4):
                                h = hg * 4 + i
                                pr, rho = h // 2, h % 2
                                ps_s = ph3ps.tile([P, TC], F32, tag="s",
                                                  bufs=4, name="ps_s")
                                nc.tensor.matmul(
                                    ps_s[:, lo:TC],
                                    kT[rho * HD:(rho + 1) * HD,
                                       pr * SB + kt * P:pr * SB + (kt + 1) * P],
                                    qT[rho * HD:(rho + 1) * HD,
                                       pr * SB + qc * TC + lo:
                                       pr * SB + (qc + 1) * TC],
                                    start=True, stop=True)
                                e = ph3.tile([P, TC], BF16, tag="e", bufs=10,
                                             name="e")
                                nc.scalar.activation(e[:, lo:TC],
                                                     ps_s[:, lo:TC],
                                                     AF.Exp,
                                                     scale=1.0 / np.sqrt(HD))
                                if d >= 0:
                                    nc.vector.tensor_mul(
                                        e[:, lo:lo + P], e[:, lo:lo + P],
                                        tri_sb[:, :])
                                if ki == 0:
                                    nc.vector.tensor_copy(eacc[i][:, :],
                                                          e[:, :])
                                else:
                                    nc.vector.tensor_add(
                                        eacc[i][:, lo:TC],
                                        eacc[i][:, lo:TC], e[:, lo:TC])
                                es.append(e)
                            return es

                        def emit_ctx(ki, es):
                            kt = kts[ki]
                            d = kt - qc * (TC // P)
                            lo = max(d, 0) * P
                            for i in range(4):
                                h = hg * 4 + i
                                pl, rho = i // 2, i % 2
                                nc.tensor.matmul(
                                    ctx_ps[pl][rho * HD:(rho + 1) * HD,
                                               lo:TC],
                                    v_sb[:, kt * HG * HD + h * HD:
                                         kt * HG * HD + (h + 1) * HD],
                                    es[i][:, lo:TC],
                                    start=(ki == 0), stop=(ki == nkt - 1))

                        prev = emit_scores(0)
                        for ki in range(1, nkt):
                            cur = emit_scores(ki)
                            emit_ctx(ki - 1, prev)
                            prev = cur
                        emit_ctx(nkt - 1, prev)
                        for i in range(4):
                            nc.tensor.matmul(
                                den_ps[32 * i:32 * i + 1, :],
                                ones_bf[:, 0:1], eacc[i][:, :],
                                start=True, stop=True,
                                tile_position=(0, 32 * i))

                        # batched reciprocal of the 4 head denominators
                        rd = ph3.tile([P, TC], F32, tag="rd", bufs=2,
                                      name="rd")
                        rd_bf = ph3.tile([P, TC], BF16, tag="rd_bf", bufs=2,
                                         name="rd_bf")
                        nc.vector.reciprocal(rd[0:97, :], den_ps[0:97, :])
                        nc.vector.tensor_copy(rd_bf[0:97, :], rd[0:97, :])
                        for i in range(4):
                            h = hg * 4 + i
                            pr, rho = h // 2, h % 2
                            pl = i // 2
                            r32 = slice(32 * i, 32 * i + 1)
                            ps_b = ph3ps.tile([P, TC], F32, tag="s", bufs=4,
                                              name="ps_b")
                            nc.tensor.matmul(ps_b[:, :],
                                             ones_rows_bf[r32, :],
                                             rd_bf[r32, :], start=True,
                                             stop=True,
                                             tile_position=(32 * i, 0))
                            rd_b = ph3.tile([P, TC], F32, tag="rd_b", bufs=2,
                                            name="rd_b")
                            nc.vector.tensor_copy(rd_b[:, :], ps_b[:, :])
                            hs = slice(rho * HD, (rho + 1) * HD)
                            nc.vector.tensor_mul(
                                ctxF[hs, pr * SB + qc * TC:
                                     pr * SB + (qc + 1) * TC],
                                ctx_ps[pl][hs, :], rd_b[hs, :])
                        if qc == 1:
                            for fco in range(hg * 4, hg * 4 + 4):
                                dense_chunk(fco, 0)
            # =========================================================
            # Phase 4: dense partial -> pairwise ReduceScatter (split in
            # two feature halves); Phase 5: LN3 + residual -> ln_in;
            # LN2 -> x2
            # =========================================================
            with tc.tile_pool(name="ph4", bufs=1) as ph4, \
                 tc.tile_pool(name="ph4ps", bufs=1, space="PSUM") as ph4ps:
                for fco in range(NFC):
                    wt = ph4.tile([P, 8 * P], BF16, tag="wd4", bufs=3,
                                  name="wd4")
                    nc.sync.dma_start(
                        out=wt[:, :], in_=wd_d[:, fco * 8 * P:(fco + 1) * 8 * P])
                    ps = ph4ps.tile([P, TC], F32, tag="d", bufs=3,
                                    name="ps_d")
                    for kc in range(8):
                        nc.tensor.matmul(
                            ps[:, :], wt[:, kc * P:(kc + 1) * P],
                            ctxF[:, kc * SB + TC:kc * SB + 2 * TC],
                            start=(kc == 0), stop=(kc == 7))
                    db = ph4.tile([P, TC], BF16, tag="db", bufs=4,
                                  name="db")
                    nc.scalar.activation(db[:, :], ps[:, :], AF.Copy)
                    fh, fr = fco // 8, fco % 8
                    nc.sync.dma_start(
                        out=rs_in[fh][P:2 * P, fr * TC:(fr + 1) * TC],
                        in_=db[:, :])
                    if fco == 7 or fco == NFC - 1:
                        fh = fco // 8
                        nc.gpsimd.collective_compute(
                            "ReduceScatter", ALU.add, replica_groups=RG_PAIR,
                            ins=[rs_in[fh][:, :].opt()],
                            outs=[rs_out[fh][:, :].opt()])
            p1_cm.__exit__(None, None, None)

            p2_cm = tc.tile_pool(name="p2", bufs=1)
            p2 = p2_cm.__enter__()
            x2 = p2.tile([P, NFC * TOK], BF16, name="x2")
            inter = p2.tile([P, NM * TC], BF16, name="inter")
            with tc.tile_pool(name="ph5", bufs=1) as ph5, \
                 tc.tile_pool(name="ph5ps", bufs=1, space="PSUM") as ph5ps:
                at = [ph5.tile([P, TC], BF16, tag=f"at{fc}", bufs=1,
                               name=f"at{fc}") for fc in range(NFC)]
                for fc in range(NFC):
                    fh, fr = fc // 8, fc % 8
                    nc.sync.dma_start(
                        out=at[fc][:, :],
                        in_=rs_out[fh][:, fr * TC:(fr + 1) * TC])
                if not zero_bias:
                    for fc in range(NFC):
                        nc.vector.tensor_scalar_add(at[fc][:, :], at[fc][:, :],
                                                    bdense[:, fc:fc + 1])
                ps_s3 = ph5ps.tile([1, TC], F32, tag="s3", bufs=1,
                                   name="ps_s3")
                ps_q3 = ph5ps.tile([1, TC], F32, tag="q3", bufs=1,
                                   name="ps_q3")
                for fc in range(NFC):
                    nc.tensor.matmul(ps_s3[:, :], ones_bf[:, 0:1],
                                     at[fc][:, :], start=(fc == 0),
                                     stop=(fc == NFC - 1))
                    sq = ph5.tile([P, TC], BF16, tag="sq", bufs=2, name="sq")
                    nc.vector.tensor_mul(sq[:, :], at[fc][:, :], at[fc][:, :])
                    nc.tensor.matmul(ps_q3[:, :], ones_bf[:, 0:1], sq[:, :],
                                     start=(fc == 0), stop=(fc == NFC - 1))
                a3_b, b23_b = _ln_rows(nc, ph5, ps_s3, ps_q3, "ln3", H)
                ps_s2 = ph5ps.tile([1, TC], F32, tag="s2", bufs=1,
                                   name="ps_s2")
                ps_q2 = ph5ps.tile([1, TC], F32, tag="q2", bufs=1,
                                   name="ps_q2")
                for fc in range(NFC):
                    sl = slice(fc * TOK, (fc + 1) * TOK)
                    t1 = ph5.tile([P, TC], F32, tag="t1", bufs=2, name="t1")
                    nc.vector.tensor_mul(t1[:, :], at[fc][:, :], a3_b[:, :])
                    nc.vector.tensor_add(t1[:, :], t1[:, :], b23_b[:, :])
                    if not ln_unit:
                        nc.vector.tensor_scalar(t1[:, :], t1[:, :],
                                                ln3w[:, fc:fc + 1],
                                                ln3b[:, fc:fc + 1],
                                                ALU.mult, ALU.add)
                    nc.vector.tensor_add(ln_in[:, sl], t1[:, :],
                                         h_res[:, sl])
                    nc.tensor.matmul(ps_s2[:, :], ones_bf[:, 0:1],
                                     ln_in[:, sl], start=(fc == 0),
                                     stop=(fc == NFC - 1))
                    sq = ph5.tile([P, TC], BF16, tag="sq", bufs=2, name="sq")
                    nc.vector.tensor_mul(sq[:, :], ln_in[:, sl], ln_in[:, sl])
                    nc.tensor.matmul(ps_q2[:, :], ones_bf[:, 0:1], sq[:, :],
                                     start=(fc == 0), stop=(fc == NFC - 1))
                a2_b, b22_b = _ln_rows(nc, ph5, ps_s2, ps_q2, "ln2", H)
                for fc in range(NFC):
                    sl = slice(fc * TOK, (fc + 1) * TOK)
                    t1 = ph5.tile([P, TC], F32, tag="t1", bufs=2, name="t1")
                    nc.vector.tensor_mul(t1[:, :], ln_in[:, sl], a2_b[:, :])
                    if ln_unit:
                        nc.vector.tensor_add(x2[:, sl], t1[:, :],
                                             b22_b[:, :])
                    else:
                        nc.vector.tensor_add(t1[:, :], t1[:, :], b22_b[:, :])
                        nc.vector.tensor_scalar(x2[:, sl], t1[:, :],
                                                ln2w[:, fc:fc + 1],
                                                ln2b[:, fc:fc + 1],
                                                ALU.mult, ALU.add)

            # =========================================================
            # Phase 6: MLP h4h + gelu -> inter; 4hh -> mlp_sb (+LN4 stats)
            # =========================================================
            with tc.tile_pool(name="ph6", bufs=1) as ph6, \
                 tc.tile_pool(name="ph6ps", bufs=1, space="PSUM") as ph6ps:
                for m in range(NM):
                    wt = ph6.tile([P, NFC * P], BF16, tag="wh", bufs=3,
                                  name="wh")
                    nc.sync.dma_start(
                        out=wt[:, :],
                        in_=w4h_d[:, m * NFC * P:(m + 1) * NFC * P])
                    ps = ph6ps.tile([P, TC], F32, tag="h", bufs=2,
                                    name="ps_h")
                    for fc in range(NFC):
                        nc.tensor.matmul(ps[:, :], wt[:, fc * P:(fc + 1) * P],
                                         x2[:, fc * TOK:(fc + 1) * TOK],
                                         start=(fc == 0),
                                         stop=(fc == NFC - 1))
                    nc.scalar.activation(inter[:, m * TC:(m + 1) * TC],
                                         ps[:, :], AF.Gelu_apprx_tanh,
                                         bias=bh4h[:, m:m + 1])
                ps_s4 = ph6ps.tile([1, TC], F32, tag="s4", bufs=1,
                                   name="ps_s4")
                ps_q4 = ph6ps.tile([1, TC], F32, tag="q4", bufs=1,
                                   name="ps_q4")
                for fco in range(NFC):
                    wt = [ph6.tile([P, NM * P // 2], BF16, tag="w4",
                                   bufs=3, name="w4")
                          for half in range(2)]
                    for half in range(2):
                        nc.sync.dma_start(
                            out=wt[half][:, :],
                            in_=w4hh_d[:, (fco * NM + half * NM // 2) * P:
                                       (fco * NM + (half + 1) * NM // 2) * P])
                    ps = ph6ps.tile([P, TC], F32, tag="f", bufs=2,
                                    name="ps_f")
                    for kc in range(NM):
                        half, kk = kc // (NM // 2), kc % (NM // 2)
                        nc.tensor.matmul(ps[:, :],
                                         wt[half][:, kk * P:(kk + 1) * P],
                                         inter[:, kc * TC:(kc + 1) * TC],
                                         start=(kc == 0),
                                         stop=(kc == NM - 1))
                    sl = slice(fco * TOK, (fco + 1) * TOK)
                    if zero_bias:
                        nc.scalar.activation(mlp_sb[:, sl], ps[:, :], AF.Copy)
                    else:
                        nc.scalar.activation(mlp_sb[:, sl], ps[:, :],
                                             AF.Identity,
                                             bias=b4hh[:, fco:fco + 1])
                    nc.tensor.matmul(ps_s4[:, :], ones_bf[:, 0:1],
                                     mlp_sb[:, sl], start=(fco == 0),
                                     stop=(fco == NFC - 1))
                    sq = ph6.tile([P, TC], BF16, tag="sq", bufs=2, name="sq")
                    nc.vector.tensor_mul(sq[:, :], mlp_sb[:, sl],
                                         mlp_sb[:, sl])
                    nc.tensor.matmul(ps_q4[:, :], ones_bf[:, 0:1], sq[:, :],
                                     start=(fco == 0), stop=(fco == NFC - 1))

                # LN4 + final residual -> out
                a4_b, b24_b = _ln_rows(nc, ph6, ps_s4, ps_q4, "ln4", H)
                for fc in range(NFC):
                    sl = slice(fc * TOK, (fc + 1) * TOK)
                    t1 = ph6.tile([P, TC], F32, tag="t1", bufs=2, name="t1")
                    nc.vector.tensor_mul(t1[:, :], mlp_sb[:, sl], a4_b[:, :])
                    nc.vector.tensor_add(t1[:, :], t1[:, :], b24_b[:, :])
                    if not ln_unit:
                        nc.vector.tensor_scalar(t1[:, :], t1[:, :],
                                                ln4w[:, fc:fc + 1],
                                                ln4b[:, fc:fc + 1],
                                                ALU.mult, ALU.add)
                    ot = ph6.tile([P, TC], F32, tag="ot", bufs=2, name="ot")
                    nc.vector.tensor_add(ot[:, :], t1[:, :], ln_in[:, sl])
                    nc.sync.dma_start(out=out_ext[fc * P:(fc + 1) * P, :],
                                      in_=ot[:, :])
            p2_cm.__exit__(None, None, None)

    nc.compile()
    return nc


def _ln_rows(nc, pool, ps_s, ps_q, name, nfeat, bbufs=1, W=TC):
    """LN row math on [1,W] stat psums -> broadcast a_b, b2_b [P,W]."""
    mu = pool.tile([1, W], F32, tag="lnr_t1", bufs=1, name=f"{name}_mu")
    m2 = pool.tile([1, W], F32, tag="lnr_t2", bufs=1, name=f"{name}_m2")
    var = pool.tile([1, W], F32, tag="lnr_t3", bufs=1, name=f"{name}_var")
    sd = pool.tile([1, W], F32, tag="lnr_t2", bufs=1, name=f"{name}_sd")
    a_row = pool.tile([1, W], F32, tag="lnr_t3", bufs=1, name=f"{name}_a")
    b2_row = pool.tile([1, W], F32, tag="lnr_t2", bufs=1, name=f"{name}_b2")
    nc.vector.tensor_scalar_mul(mu[:, :], ps_s[:, :], 1.0 / nfeat)
    nc.vector.tensor_scalar_mul(m2[:, :], ps_q[:, :], 1.0 / nfeat)
    nc.vector.tensor_mul(var[:, :], mu[:, :], mu[:, :])
    nc.vector.tensor_sub(var[:, :], m2[:, :], var[:, :])
    nc.scalar.activation(sd[:, :], var[:, :], AF.Sqrt, bias=EPS)
    nc.vector.reciprocal(a_row[:, :], sd[:, :])
    nc.vector.tensor_mul(b2_row[:, :], mu[:, :], a_row[:, :])
    nc.vector.tensor_scalar_mul(b2_row[:, :], b2_row[:, :], -1.0)
    a_b = pool.tile([P, W], F32, tag="lnr_ab", bufs=bbufs, name=f"{name}_ab")
    b2_b = pool.tile([P, W], F32, tag="lnr_b2b", bufs=bbufs,
                     name=f"{name}_b2b")
    nc.gpsimd.partition_broadcast(a_b[:, :], a_row[:, :])
    nc.gpsimd.partition_broadcast(b2_b[:, :], b2_row[:, :])
    return a_b, b2_b


# ----------------------------------------------------------------------
_cache = {}


def _get_program(mask_np, zero_bv, zero_bias, ln_unit):
    key = (mask_np.tobytes(), zero_bv, zero_bias, ln_unit)
    kh = hash(key)
    if kh not in _cache:
        _cache[kh] = build_program(_causal_block_status(mask_np), zero_bv,
                                   zero_bias, ln_unit)
    return _cache[kh]


def kernel(hidden_states, mask, ln1_w, ln1_b, w_qkv, b_qkv, w_dense, b_dense,
           ln3_w, ln3_b, ln2_w, ln2_b, w_h4h, b_h4h, w_4hh, b_4hh,
           ln4_w, ln4_b):
    hidden_states = np.asarray(hidden_states, np.float32)
    mask2d = np.asarray(mask, np.float32).reshape(S, S)
    w_qkv = np.asarray(w_qkv, np.float32)
    b_qkv = np.asarray(b_qkv, np.float32)
    w_dense = np.asarray(w_dense, np.float32)
    b_dense = np.asarray(b_dense, np.float32)
    w_h4h = np.asarray(w_h4h, np.float32)
    b_h4h = np.asarray(b_h4h, np.float32)
    w_4hh = np.asarray(w_4hh, np.float32)
    b_4hh = np.asarray(b_4hh, np.float32)

    zero_bv = bool(np.all(b_qkv[2 * H:] == 0.0))
    zero_bias = bool(np.all(b_qkv[:2 * H] == 0.0)
                     and np.all(b_dense == 0.0)
                     and np.all(b_4hh == 0.0))
    ln_unit = bool(all(np.all(np.asarray(w, np.float32) == 1.0)
                       for w in (ln1_w, ln2_w, ln3_w, ln4_w))
                   and all(np.all(np.asarray(b, np.float32) == 0.0)
                           for b in (ln1_b, ln2_b, ln3_b, ln4_b)))
    prog = _get_program(mask2d, zero_bv, zero_bias, ln_unit)

    block_status = _causal_block_status(mask2d)
    masked_blocks = sorted(k for k, v in block_status.items()
                           if v == "masked")
    # verify the causal staircase structure the kernel assumes: each
    # masked block (kt, qc) sits on the diagonal band (d = kt - 4*qc in
    # 0..3) with sub-chunks j<d zero, j>d one, j==d the 128x128 triangle
    mt = mask2d.T
    tri = mt[0:P, 0:P]
    for (kt, qc) in masked_blocks:
        d = kt - qc * (TC // P)
        assert 0 <= d < TC // P, f"non-causal masked block {(kt, qc)}"
        blk = mt[kt * P:(kt + 1) * P, qc * TC:(qc + 1) * TC]
        for j in range(TC // P):
            sub = blk[:, j * P:(j + 1) * P]
            if j < d:
                assert np.all(sub == 0.0)
            elif j > d:
                assert np.all(sub == 1.0)
            else:
                assert np.array_equal(sub, tri)
    mask_pack = np.ascontiguousarray(tri).astype(bf16)

    # shared weight packs (same for all cores)
    w4h_pack = np.ascontiguousarray(
        w_h4h.reshape(NFC, P, NM, P).transpose(1, 2, 0, 3)
        .reshape(P, NM * NFC * P)).astype(bf16)
    w4hh_pack = np.ascontiguousarray(
        w_4hh.reshape(NM, P, NFC, P).transpose(1, 2, 0, 3)
        .reshape(P, NFC * NM * P)).astype(bf16)

    # per-head-group (j) packs
    wqk_packs, wv_packs, wd_packs, bqk_cols, bv_rows = [], [], [], [], []
    for j in range(2):
        qo, ko, vo = j * 1024, H + j * 1024, 2 * H + j * 1024
        wq = w_qkv[:, qo:qo + 1024]      # [2048, 1024]
        wk = w_qkv[:, ko:ko + 1024]
        wv = w_qkv[:, vo:vo + 1024]
        wqk = np.concatenate([wq, wk], axis=1)   # m-chunks 0..15
        wqk_packs.append(np.ascontiguousarray(
            wqk.reshape(NFC, P, 16, P).transpose(1, 2, 0, 3)
            .reshape(P, 16 * NFC * P)).astype(bf16))
        # wv chunk (vf, fc) at [:, (vf*NFC+fc)*TC : ...]
        wv_packs.append(np.ascontiguousarray(
            wv.reshape(NFC, P, 2, TC).transpose(1, 2, 0, 3)
            .reshape(P, 2 * NFC * TC)).astype(bf16))

        wd = w_dense[j * 1024:(j + 1) * 1024, :]  # [1024, 2048]
        wd_packs.append(np.ascontiguousarray(
            wd.reshape(8, P, NFC, P).transpose(1, 2, 0, 3)
            .reshape(P, NFC * 8 * P)).astype(bf16))
        bq = np.concatenate([b_qkv[qo:qo + 1024], b_qkv[ko:ko + 1024]])
        bqk_cols.append(bq.reshape(16, P).T)     # [128, 16]
        bv_rows.append(b_qkv[vo:vo + 1024].reshape(1, 1024))

    def col16(v):
        return v.reshape(NFC, P).T               # [128, 16]

    cpacks = []
    for j in range(2):
        cp = np.zeros((P, 240), np.float32)
        cp[:, 0:16] = col16(np.asarray(ln1_w, np.float32))
        cp[:, 16:32] = col16(np.asarray(ln1_b, np.float32))
        cp[:, 32:48] = col16(np.asarray(ln2_w, np.float32))
        cp[:, 48:64] = col16(np.asarray(ln2_b, np.float32))
        cp[:, 64:80] = col16(np.asarray(ln3_w, np.float32))
        cp[:, 80:96] = col16(np.asarray(ln3_b, np.float32))
        cp[:, 96:112] = col16(np.asarray(ln4_w, np.float32))
        cp[:, 112:128] = col16(np.asarray(ln4_b, np.float32))
        cp[:, 128:144] = bqk_cols[j]
        cp[:, 144:208] = b_h4h.reshape(NM, P).T
        cp[:, 208:224] = col16(b_4hh)
        cp[:, 224:240] = col16(b_dense)
        cpacks.append(cp)

    h_batches = []
    for b in range(B):
        h_batches.append(np.ascontiguousarray(
            hidden_states[b].T).astype(bf16))    # [2048, 1024] bf16

    in_maps = []
    for c in range(NC):
        b, j = c // 2, c % 2
        h_own = np.ascontiguousarray(
            hidden_states[b, j * TOK:(j + 1) * TOK, :].T).astype(bf16)
        im = {
            "h_batch": h_batches[b],
            "h_own": h_own,
            "cpack": cpacks[j],
            "b_v": bv_rows[j],
            "wqk": wqk_packs[j],
            "wv": wv_packs[j],
            "wd": wd_packs[j],
            "w4h": w4h_pack,
            "w4hh": w4hh_pack,
            "maskb": mask_pack,
        }
        in_maps.append(im)

    res = run_bass_kernel_spmd(prog, in_maps, core_ids=list(range(NC)))
    out = np.empty((B, S, H), np.float32)
    for c in range(NC):
        b, j = c // 2, c % 2
        out[b, j * TOK:(j + 1) * TOK, :] = res.results[c]["out"].T
    return out
